# revision 1
# baseline (speedup 1.0000x reference)
"""GCN (2-layer GCNConv + mean-pool + linear) on 8 Trainium2 NeuronCores.

Strategy (feature-major, scan-based segment reduction):
  - dst-shard nodes across 8 cores (12544 each, padded to 100352); edges live on
    their dst core, grouped into 8 gpsimd groups by src chunk, dst-sorted within.
  - y = dinv * x built per core, exchanged via AllGather into an SBUF-resident
    feature-major table [128, 12544] (group k rows 16k+f = y^T[f, chunk k]).
  - per chunk: gpsimd ap_gather -> per-edge src features; * edge weight; DVE
    prefix scan along edges; extract per-node run boundaries (indirect_copy);
    diff -> per-group partial sums; merge groups with a PE selector matmul.
  - layer 2 propagates one scalar per node: mean_pool(A h W2) @ Wlin
    == mean_pool(A (h (W2 @ Wlin))), so only z = h1 @ (W2@Wlin) flows.
  - pooling: prefix scan of q^T + extraction at graph ends; AllReduce [256].
All floating-point math runs on device; the host only sorts/permutes indices,
pads with zeros/ones, and builds constant 0/1 selector matrices.
"""
import numpy as np

N = 100000
NC = 8
NPC = 12544
NPAD = NC * NPC
NBLK = 98
S = 14
M = NPC // S          # 896
B = 256
F = 10
ECOLS = -(-(M + 1) // 16)  # 57
NPOOL = 64


def _ceil16(v):
    return ((v + 15) // 16) * 16


def _wrap16(vals, ncols, pad=0):
    v = np.asarray(vals)
    buf = np.full(ncols * 16, pad, dtype=v.dtype if len(v) else np.int16)
    buf[: len(v)] = v
    return buf.reshape(ncols, 16).T.copy()


def prep(x, edge_index, edge_weight, batch):
    """Pure index/permutation prep. Returns (per-core input dicts, meta)."""
    src = np.asarray(edge_index[0], dtype=np.int64)
    dst = np.asarray(edge_index[1], dtype=np.int64)
    w = np.asarray(edge_weight, dtype=np.float32)
    batch = np.asarray(batch, dtype=np.int64)
    x = np.asarray(x, dtype=np.float32)

    loops = np.arange(N, dtype=np.int64)
    src_all = np.concatenate([src, loops])
    dst_all = np.concatenate([dst, loops])
    w_all = np.concatenate([w, np.ones(N, dtype=np.float32)])

    g_all = src_all // NPC
    core_all = dst_all // NPC
    chunk_all = (dst_all % NPC) // M
    cell = ((core_all * NC + g_all) * S + chunk_all).astype(np.int64)
    C_ch = _ceil16(int(np.bincount(cell, minlength=NC * NC * S).max()) + 2)
    DP = int(np.bincount(dst_all, minlength=N).max())

    cnt = np.maximum(np.bincount(batch, minlength=B), 1).astype(np.int32)

    # constant 0/1 selector matrices (structural, data-independent)
    sel = np.zeros((128, 16), dtype=np.float32)
    sel[np.arange(128), np.arange(128) % 16] = 1.0
    sel2 = np.zeros((128, 1), dtype=np.float32)
    sel2[::16, 0] = 1.0

    cores = []
    for c in range(NC):
        lo = c * NPC
        hi = min((c + 1) * NPC, N)
        nreal = hi - lo

        xpk = np.zeros((128, NBLK * 11), dtype=np.float32)
        xl = np.zeros((NPC, F), dtype=np.float32)
        xl[:nreal] = x[lo:hi]
        for b_ in range(NBLK):
            xpk[:, b_ * 11 + 1 : b_ * 11 + 1 + F] = xl[b_ * 128 : (b_ + 1) * 128]

        mask = (dst_all >= lo) & (dst_all < hi)
        es, ed, ew = src_all[mask], (dst_all[mask] - lo).astype(np.int64), w_all[mask]
        eg = es // NPC
        esl = (es - eg * NPC).astype(np.int16)

        w2pad = np.zeros((128, NBLK * DP), dtype=np.float32)
        order_d = np.argsort(ed, kind="stable")
        d_sorted, w_sorted = ed[order_d], ew[order_d]
        node_starts = np.searchsorted(d_sorted, np.arange(NPC + 1))
        p_of = np.arange(NPC) % 128
        b_of = np.arange(NPC) // 128
        lens = np.diff(node_starts)
        for l in np.nonzero(lens)[0]:
            a = node_starts[l]
            w2pad[p_of[l], b_of[l] * DP : b_of[l] * DP + lens[l]] = w_sorted[a : a + lens[l]]
        for l in range(nreal, NPC):  # pad nodes: deg = 1 so dinv stays finite
            w2pad[p_of[l], b_of[l] * DP] = 1.0

        gidx = np.zeros((128, S * (C_ch // 16)), dtype=np.int16)
        wrep = np.zeros((128, S * C_ch), dtype=np.float32)
        eidx = np.zeros((128, S * ECOLS), dtype=np.uint16)
        order = np.lexsort((ed, eg))
        gs, ds, ws, sls = eg[order], ed[order], ew[order], esl[order]
        grp_starts = np.searchsorted(gs, np.arange(NC + 1))
        for k in range(NC):
            ga, gb = grp_starts[k], grp_starts[k + 1]
            dk, wk, slk = ds[ga:gb], ws[ga:gb], sls[ga:gb]
            chunk_starts = np.searchsorted(dk, np.arange(0, NPC + M, M))
            for j in range(S):
                ca, cb = chunk_starts[j], chunk_starts[j + 1]
                n_e = cb - ca
                idx_slots = np.zeros(C_ch, dtype=np.int16)
                idx_slots[1 : 1 + n_e] = slk[ca:cb]
                w_slots = np.zeros(C_ch, dtype=np.float32)
                w_slots[1 : 1 + n_e] = wk[ca:cb]
                gidx[16 * k : 16 * (k + 1), j * (C_ch // 16) : (j + 1) * (C_ch // 16)] = (
                    idx_slots.reshape(C_ch // 16, 16).T
                )
                wrep[16 * k : 16 * (k + 1), j * C_ch : (j + 1) * C_ch] = w_slots[None, :]
                ends = np.zeros(M + 1, dtype=np.uint16)
                ends[1:] = np.searchsorted(dk[ca:cb], np.arange(j * M, (j + 1) * M), side="right").astype(np.uint16)
                epad = np.zeros(ECOLS * 16, dtype=np.uint16)
                epad[: M + 1] = ends
                eidx[16 * k : 16 * (k + 1), j * ECOLS : (j + 1) * ECOLS] = epad.reshape(ECOLS, 16).T

        gmin, gmax = int(batch[lo]), int(batch[hi - 1])
        glist = np.arange(gmin, gmax + 1)
        gends = np.searchsorted(batch, glist, side="right") - lo
        gends = np.minimum(gends, NPC).astype(np.int16)
        n_gc = len(glist)
        pool_end_vals = np.full(NPOOL, gends[-1] - 1, dtype=np.int16)
        pool_end_vals[:n_gc] = gends - 1
        pool_end = _wrap16(pool_end_vals, 4)
        place = np.full(B, NPOOL, dtype=np.int16)
        place[gmin : gmax + 1] = np.arange(n_gc, dtype=np.int16)
        pool_place = _wrap16(place, 16)

        cores.append(
            dict(
                xpk=xpk, w2pad=w2pad, gidx=gidx, wrep=wrep, eidx=eidx,
                pool_end=pool_end.astype(np.int16), pool_place=pool_place.astype(np.int16),
                cnt=cnt.reshape(1, B), sel=sel, sel2=sel2,
            )
        )
    return cores, dict(C_ch=C_ch, DP=DP)


# ------------------------------------------------------------------ device
def build_program(C_ch, DP):
    import concourse.bass as bass
    import concourse.bacc as bacc
    import concourse.mybir as mybir
    import concourse.tile as tile
    from concourse.masks import make_identity

    f32 = mybir.dt.float32
    i16 = mybir.dt.int16
    u16 = mybir.dt.uint16
    i32 = mybir.dt.int32
    AX = mybir.AxisListType.X
    OP = mybir.AluOpType
    AF = mybir.ActivationFunctionType

    nc = bacc.Bacc("TRN2", target_bir_lowering=False, debug=False, num_devices=NC)

    def din(name, shape, dt=f32):
        return nc.dram_tensor(name, shape, dt, kind="ExternalInput")

    xpk_d = din("xpk", [128, NBLK * 11])
    w2_d = din("w2pad", [128, NBLK * DP])
    gidx_d = din("gidx", [128, S * (C_ch // 16)], i16)
    wrep_d = din("wrep", [128, S * C_ch])
    eidx_d = din("eidx", [128, S * ECOLS], u16)
    pend_d = din("pool_end", [16, 4], i16)
    pplace_d = din("pool_place", [16, 16], i16)
    cnt_d = din("cnt", [1, B], i32)
    sel_d = din("sel", [128, 16])
    sel2_d = din("sel2", [128, 1])
    W1_d = din("W1", [F, 128])
    b1_d = din("b1", [128, 1])
    W2_d = din("W2", [128, 128])
    wlr_d = din("wlin_row", [1, 128])
    wlc_d = din("wlin_col", [128, 1])
    blin_d = din("blin", [1, 1])
    b2_d = din("b2row", [1, 128])
    out_d = nc.dram_tensor("out", [1, B], f32, kind="ExternalOutput")

    rg = [list(range(NC))]

    with tile.TileContext(nc) as tc:
        from contextlib import ExitStack

        with ExitStack() as ctx:
            sb = ctx.enter_context(tc.tile_pool(name="sb", bufs=1))
            big = ctx.enter_context(tc.tile_pool(name="big", bufs=1))
            dram = ctx.enter_context(tc.tile_pool(name="dram", bufs=1, space="DRAM"))
            gpool = ctx.enter_context(tc.tile_pool(name="gp", bufs=2))
            wpool = ctx.enter_context(tc.tile_pool(name="wp", bufs=1))
            mpool = ctx.enter_context(tc.tile_pool(name="mp", bufs=1))
            epool = ctx.enter_context(tc.tile_pool(name="ep", bufs=2))
            ppool = ctx.enter_context(tc.tile_pool(name="pp", bufs=2))
            tpool = ctx.enter_context(tc.tile_pool(name="tp", bufs=1))
            hpool = ctx.enter_context(tc.tile_pool(name="hp", bufs=2))
            dpool = ctx.enter_context(tc.tile_pool(name="dp", bufs=1))

            # --- constants
            selt = sb.tile([128, 16], f32)
            nc.sync.dma_start(out=selt[:], in_=sel_d[:, :])
            sel2t = sb.tile([128, 1], f32)
            nc.sync.dma_start(out=sel2t[:], in_=sel2_d[:, :])
            W1t = sb.tile([F, 128], f32)
            nc.sync.dma_start(out=W1t[:], in_=W1_d[:, :])
            b1t = sb.tile([128, 1], f32)
            nc.sync.dma_start(out=b1t[:], in_=b1_d[:, :])
            wlrt = sb.tile([1, 128], f32)
            nc.sync.dma_start(out=wlrt[:], in_=wlr_d[:, :])
            wlct = sb.tile([128, 1], f32)
            nc.sync.dma_start(out=wlct[:], in_=wlc_d[:, :])
            blint = sb.tile([1, 1], f32)
            nc.sync.dma_start(out=blint[:], in_=blin_d[:, :])
            b2t = sb.tile([1, 128], f32)
            nc.sync.dma_start(out=b2t[:], in_=b2_d[:, :])
            pendt = sb.tile([16, 4], i16)
            nc.sync.dma_start(out=pendt[:], in_=pend_d[:, :])
            pplacet = sb.tile([16, 16], i16)
            nc.sync.dma_start(out=pplacet[:], in_=pplace_d[:, :])
            cntt = sb.tile([1, B], i32)
            nc.sync.dma_start(out=cntt[:], in_=cnt_d[:, :])
            zerot = sb.tile([128, 1], f32)
            nc.vector.memset(zerot[:], 0.0)
            ones10 = sb.tile([1, 16], f32)
            nc.vector.memset(ones10[:], 1.0)

            # --- phase A: deg, dinv, y (in place on xpk), transposes
            wz = sb.tile([128, 1], f32)
            with tc.tile_pool(name="pha", bufs=1) as pha, \
                 tc.tile_pool(name="pst", bufs=2, space="PSUM") as pst:
                # wz = W2 @ Wlin via PE: transpose W2, then (W2^T).T @ wlin_col
                ident = pha.tile([128, 128], f32)
                make_identity(nc, ident[:])
                W2t = pha.tile([128, 128], f32)
                nc.sync.dma_start(out=W2t[:], in_=W2_d[:, :])
                w2tp = pst.tile([128, 512], f32, tag="pt")
                nc.tensor.transpose(out=w2tp[:, :128], in_=W2t[:], identity=ident[:])
                w2ts = pha.tile([128, 128], f32)
                nc.scalar.copy(out=w2ts[:], in_=w2tp[:, :128])
                wzp = pst.tile([128, 512], f32, tag="pt")
                nc.tensor.matmul(out=wzp[:, :1], lhsT=w2ts[:], rhs=wlct[:], start=True, stop=True)
                nc.scalar.copy(out=wz[:], in_=wzp[:, :1])
                xpkt = pha.tile([128, NBLK * 11], f32)
                nc.sync.dma_start(out=xpkt[:], in_=xpk_d[:, :])
                deg = pha.tile([128, NBLK], f32)
                HB = NBLK // 7
                for hh in range(7):
                    w2t_ = pha.tile([128, HB * DP], f32, tag="w2t", name=f"w2t{hh}")
                    nc.sync.dma_start(out=w2t_[:], in_=w2_d[:, hh * HB * DP : (hh + 1) * HB * DP])
                    nc.vector.tensor_reduce(
                        out=deg[:, hh * HB : (hh + 1) * HB],
                        in_=w2t_[:].rearrange("p (b d) -> p b d", d=DP), axis=AX, op=OP.add
                    )
                dinv = pha.tile([128, NBLK], f32)
                nc.scalar.activation(out=deg[:], in_=deg[:], func=AF.Sqrt)
                nc.vector.reciprocal(out=dinv[:], in_=deg[:])
                xv = xpkt[:].rearrange("p (b f) -> p b f", f=11)
                dv = dinv[:].rearrange("p (b o) -> p b o", o=1)
                nc.vector.tensor_tensor(
                    out=xv[:, :, 1 : F + 1], in0=xv[:, :, 1 : F + 1],
                    in1=dv.to_broadcast([128, NBLK, F]), op=OP.mult
                )
                nc.vector.tensor_copy(out=xv[:, :, 0:1], in_=dv)

                glob = big.tile([128, NPC], f32)
                nc.scalar.activation(
                    out=glob[0:16, :], in_=zerot[0:16, :].to_broadcast([16, NPC]), func=AF.Copy
                )
                for b4 in range(25):
                    nb = min(4, NBLK - b4 * 4)
                    ptile = pst.tile([128, 512], f32, tag="pt", name=f"ptile{b4}")
                    for bb in range(nb):
                        b_ = b4 * 4 + bb
                        nc.tensor.transpose(
                            out=ptile[0:11, bb * 128 : (bb + 1) * 128],
                            in_=xv[:, b_, :],
                            identity=ident[:],
                        )
                    nc.scalar.copy(
                        out=glob[0:11, b4 * 512 : b4 * 512 + nb * 128],
                        in_=ptile[0:11, : nb * 128],
                    )

            # --- AllGather y
            yag_in = dram.tile([F, NPC], f32)
            yag_out = dram.tile([NC, F * NPC], f32)
            nc.sync.dma_start(out=yag_in[:], in_=glob[1 : F + 1, :])
            nc.gpsimd.collective_compute(
                "AllGather", mybir.AluOpType.bypass, replica_groups=rg,
                ins=[yag_in[:]], outs=[yag_out[:]],
            )
            table = big.tile([128, NPC], f32)
            nc.scalar.activation(
                out=table[:], in_=zerot[:].to_broadcast([128, NPC]), func=AF.Copy
            )
            yag_v = yag_out[:].rearrange("k (f n) -> k f n", f=F)
            for k in range(NC):
                nc.sync.dma_start(out=table[16 * k : 16 * k + F, :], in_=yag_v[k])

            psm = ctx.enter_context(tc.tile_pool(name="psm", bufs=2, space="PSUM"))
            psb = ctx.enter_context(tc.tile_pool(name="psb", bufs=2, space="PSUM"))

            # --- shared chunk pipeline (layers 1 and 2); gathers batched per
            # chunk-PAIR (halves Pool-engine gather dispatches)
            GC = C_ch // 16

            WSTART = {0: 4, 4: 4, 8: 4, 12: 2}  # chunk windows for batched gathers

            def edge_win(j0, nchunk, lyr):
                gix = gpool.tile([128, 4 * GC], i16, tag="gix", name=f"gix{lyr}_{j0}")
                nc.sync.dma_start(out=gix[:, : nchunk * GC], in_=gidx_d[:, j0 * GC : (j0 + nchunk) * GC])
                msgs = mpool.tile([128, 4 * C_ch], f32, tag="msgs", name=f"msgs{lyr}_{j0}")
                nc.gpsimd.ap_gather(
                    out_ap=msgs[:, : nchunk * C_ch], in_ap=table[:], idxs_ap=gix[:, : nchunk * GC],
                    channels=128, num_elems=NPC, d=1, num_idxs=nchunk * C_ch,
                )
                for hw_ in range(nchunk // 2):
                    wre = wpool.tile([128, 2 * C_ch], f32, tag="wre", name=f"wre{lyr}_{j0}_{hw_}")
                    nc.sync.dma_start(
                        out=wre[:],
                        in_=wrep_d[:, (j0 + 2 * hw_) * C_ch : (j0 + 2 * hw_ + 2) * C_ch],
                    )
                    nc.vector.tensor_tensor(
                        out=msgs[:, 2 * hw_ * C_ch : (2 * hw_ + 2) * C_ch],
                        in0=msgs[:, 2 * hw_ * C_ch : (2 * hw_ + 2) * C_ch],
                        in1=wre[:], op=OP.mult,
                    )
                return msgs

            def chunk_tail(msgs, js, j):
                sc = msgs[:, js * C_ch : (js + 1) * C_ch]
                nc.vector.tensor_tensor_scan(
                    out=sc, data0=sc,
                    data1=zerot[:].to_broadcast([128, C_ch]),
                    initial=0.0, op0=OP.add, op1=OP.add,
                )
                eix = epool.tile([128, ECOLS], u16, tag="eix")
                nc.sync.dma_start(out=eix[:], in_=eidx_d[:, j * ECOLS : (j + 1) * ECOLS])
                E = epool.tile([128, ECOLS * 16], f32, tag="E")
                nc.gpsimd.indirect_copy(
                    out=E[:, : M + 1], data=sc, idxs=eix[:], i_know_ap_gather_is_preferred=True
                )
                Pd = ppool.tile([128, M], f32, tag="Pd")
                nc.vector.tensor_tensor(out=Pd[:], in0=E[:, 1 : M + 1], in1=E[:, 0:M], op=OP.subtract)
                return Pd

            # --- layer 1
            jw1 = 0
            for j in range(S):
                if j in WSTART:
                    msgs_l1 = edge_win(j, WSTART[j], 1)
                    jw1 = j
                Pd = chunk_tail(msgs_l1, j - jw1, j)
                pT = tpool.tile([F, M], f32, tag="pT")
                for h in range(2):
                    pm_ = psm.tile([F, 448], f32, tag="pm")
                    nc.tensor.matmul(
                        out=pm_[:], lhsT=selt[:, 0:F], rhs=Pd[:, h * 448 : (h + 1) * 448],
                        start=True, stop=True,
                    )
                    dvp = psb.tile([F, 448], f32, tag="dvp", name=f"dvp{j}_{h}")
                    nc.tensor.matmul(
                        out=dvp[:], lhsT=ones10[:, 0:F],
                        rhs=glob[0:1, j * M + h * 448 : j * M + (h + 1) * 448],
                        start=True, stop=True,
                    )
                    dvs = dpool.tile([F, 448], f32, tag="dvs", name=f"dvs{j}_{h}")
                    nc.scalar.copy(out=dvs[:], in_=dvp[:])
                    nc.vector.tensor_tensor(
                        out=pT[:, h * 448 : (h + 1) * 448], in0=pm_[:], in1=dvs[:], op=OP.mult,
                    )
                zps = [psb.tile([1, 512], f32, tag="zp", name=f"zp{j}_{h_}") for h_ in range(2)]
                for bb in range(7):
                    st = psm.tile([128, 128], f32, tag="st")
                    nc.tensor.matmul(
                        out=st[:], lhsT=W1t[:], rhs=pT[:, bb * 128 : (bb + 1) * 128],
                        start=True, stop=True,
                    )
                    ht = hpool.tile([128, 128], f32, tag="ht")
                    nc.scalar.activation(out=ht[:], in_=st[:], func=AF.Relu, bias=b1t[:])
                    zp = zps[bb // 4]
                    nc.tensor.matmul(
                        out=zp[:, (bb % 4) * 128 : (bb % 4) * 128 + 128],
                        lhsT=wz[:], rhs=ht[:], start=True, stop=True,
                    )
                for h, n_ in ((0, 512), (1, 384)):
                    nc.vector.tensor_tensor(
                        out=glob[32:33, j * M + h * 512 : j * M + h * 512 + n_],
                        in0=zps[h][:, :n_],
                        in1=glob[0:1, j * M + h * 512 : j * M + h * 512 + n_],
                        op=OP.mult,
                    )

            # --- AllGather zy; z-table rows 16k
            zag_in = dram.tile([1, NPC], f32)
            zag_out = dram.tile([NC, NPC], f32)
            nc.sync.dma_start(out=zag_in[:], in_=glob[32:33, :])
            nc.gpsimd.collective_compute(
                "AllGather", mybir.AluOpType.bypass, replica_groups=rg,
                ins=[zag_in[:]], outs=[zag_out[:]],
            )
            for k in range(NC):
                nc.sync.dma_start(out=table[16 * k : 16 * k + 1, :], in_=zag_out[k : k + 1, :])

            # --- layer 2
            jw2 = 0
            for j in range(S):
                if j in WSTART:
                    msgs_l2 = edge_win(j, WSTART[j], 2)
                    jw2 = j
                Pd = chunk_tail(msgs_l2, j - jw2, j)
                for h in range(2):
                    n_ = 448 if h == 0 else 448
                    qm = psb.tile([1, 512], f32, tag="zp", name=f"qm{j}_{h}")
                    nc.tensor.matmul(
                        out=qm[:, :448], lhsT=sel2t[:], rhs=Pd[:, h * 448 : (h + 1) * 448],
                        start=True, stop=True,
                    )
                    nc.vector.tensor_tensor(
                        out=glob[64:65, j * M + h * 448 : j * M + (h + 1) * 448],
                        in0=qm[:, :448],
                        in1=glob[0:1, j * M + h * 448 : j * M + (h + 1) * 448],
                        op=OP.mult,
                    )

            # --- pooling
            nc.vector.tensor_tensor_scan(
                out=glob[0:1, :], data0=glob[64:65, :],
                data1=zerot[64:65, :].to_broadcast([1, NPC]),
                initial=0.0, op0=OP.add, op1=OP.add,
            )
            Ep = sb.tile([16, NPOOL], f32)
            nc.gpsimd.ap_gather(
                out_ap=Ep[:], in_ap=glob[0:16, :], idxs_ap=pendt[:],
                channels=16, num_elems=NPC, d=1, num_idxs=NPOOL,
            )
            Pp = sb.tile([16, NPOOL + 1], f32)
            nc.vector.memset(Pp[:], 0.0)
            nc.vector.tensor_copy(out=Pp[0:1, 0:1], in_=Ep[0:1, 0:1])
            nc.vector.tensor_tensor(
                out=Pp[0:1, 1:NPOOL], in0=Ep[0:1, 1:NPOOL], in1=Ep[0:1, 0 : NPOOL - 1],
                op=OP.subtract,
            )
            placed = sb.tile([16, B], f32)
            nc.gpsimd.ap_gather(
                out_ap=placed[:], in_ap=Pp[:], idxs_ap=pplacet[:],
                channels=16, num_elems=NPOOL + 1, d=1, num_idxs=B,
            )

            par_in = dram.tile([1, B], f32)
            par_out = dram.tile([1, B], f32)
            nc.sync.dma_start(out=par_in[:], in_=placed[0:1, :])
            nc.gpsimd.collective_compute(
                "AllReduce", mybir.AluOpType.add, replica_groups=rg,
                ins=[par_in[:]], outs=[par_out[:]],
            )
            art = sb.tile([1, B], f32)
            nc.sync.dma_start(out=art[:], in_=par_out[:])

            cntf = sb.tile([1, B], f32)
            nc.vector.tensor_copy(out=cntf[:], in_=cntt[:])
            rec = sb.tile([1, B], f32)
            nc.vector.reciprocal(out=rec[:], in_=cntf[:])
            res = sb.tile([1, B], f32)
            nc.vector.tensor_tensor(out=res[:], in0=art[:], in1=rec[:], op=OP.mult)
            cb = sb.tile([1, 128], f32)
            nc.vector.tensor_tensor(out=cb[:], in0=b2t[:], in1=wlrt[:], op=OP.mult)
            cs = sb.tile([1, 1], f32)
            nc.vector.tensor_reduce(out=cs[:], in_=cb[:], axis=AX, op=OP.add)
            nc.vector.tensor_tensor(out=cs[:], in0=cs[:], in1=blint[:], op=OP.add)
            nc.vector.tensor_tensor(
                out=res[:], in0=res[:], in1=cs[:].to_broadcast([1, B]), op=OP.add
            )
            nc.sync.dma_start(out=out_d[:, :], in_=res[:])

    nc.compile()
    return nc


_CACHE = {}


def kernel(**inputs):
    from concourse.bass_utils import run_bass_kernel_spmd

    cores, meta = prep(
        inputs["x"], inputs["edge_index"], inputs["edge_weight"], inputs["batch"]
    )
    key = (meta["C_ch"], meta["DP"])
    if key not in _CACHE:
        _CACHE[key] = build_program(*key)
    nc = _CACHE[key]

    W1 = np.asarray(inputs["W1"], dtype=np.float32)
    b1 = np.asarray(inputs["b1"], dtype=np.float32).reshape(128, 1)
    W2 = np.asarray(inputs["W2"], dtype=np.float32)
    wlr = np.asarray(inputs["Wlin"], dtype=np.float32).reshape(1, 128)
    wlc = np.asarray(inputs["Wlin"], dtype=np.float32).reshape(128, 1)
    blin = np.asarray(inputs["blin"], dtype=np.float32).reshape(1, 1)
    b2r = np.asarray(inputs["b2"], dtype=np.float32).reshape(1, 128)

    in_maps = []
    for c in range(NC):
        cr = cores[c]
        in_maps.append(
            dict(
                xpk=cr["xpk"], w2pad=cr["w2pad"], gidx=cr["gidx"], wrep=cr["wrep"],
                eidx=cr["eidx"], pool_end=cr["pool_end"], pool_place=cr["pool_place"],
                cnt=cr["cnt"], sel=cr["sel"], sel2=cr["sel2"],
                W1=W1, b1=b1, W2=W2, wlin_row=wlr, wlin_col=wlc, blin=blin, b2row=b2r,
            )
        )
    res = run_bass_kernel_spmd(nc, in_maps, list(range(NC)))
    out = np.asarray(res.results[0]["out"], dtype=np.float32).reshape(B, 1)
    return out



# revision 15
# speedup vs baseline: 1.5399x; 1.5399x over previous
"""GCN (2-layer GCNConv + mean-pool + linear) on 8 Trainium2 NeuronCores.

v2: fp16 data path, scan-based segment reduction with shared slot structure
across both layers.
  - dst-shard nodes across 8 cores (12544 each); edges (NO self-loops) grouped
    into 8 gpsimd groups by src chunk, dst-sorted within (group, chunk) cells.
  - self-loop contributions applied via PE-accumulate matmuls against the
    gathered feature table (one-hot selectors supplied per core).
  - y = dinv*x built per core (fp16), AllGathered (fp16) into an SBUF table
    [128, 12544] (group k rows 16k+f). Layer 2 overwrites rows 16k with zy.
  - per 7-chunk window: one ap_gather -> fp16 msgs; one 4x DVE mult by fp16
    edge weights; per chunk: in-place fp16 scan, indirect_copy boundary
    extract, diff -> Pd; PE merges groups (+ self loop), dinv scale, W1+relu,
    z = h @ (W2@Wlin); layer 2 reduces the same slots on scalar zy.
  - wre/gidx/eidx identical across layers: loaded once, reused.
  - pooling: per-128-node-block PE transpose-merge -> q98 [128, 98], graph
    sums via 0/1 indicator matmuls, AllGather [8,64] + 2-gather combine on
    core 0.
Host does only index/permutation/padding work and dtype conversion.
"""
import numpy as np

N = 100000
NC = 8
NPC = 12544
NPAD = NC * NPC
NBLK = 98
S = 14
M = NPC // S          # 896
B = 256
F = 10
ECOLS = 64  # ceil((M+1)/16)=57, padded to 64 for 4B-aligned slices
GCOL = 64             # graph slots per core
GTOT = 520            # combine table width (8*64 + zero pad)
WINS = ((0, 7), (7, 7))


def _ceil16(v):
    return ((v + 15) // 16) * 16


def _wrap16(vals, ncols, pad=0):
    v = np.asarray(vals)
    buf = np.full(ncols * 16, pad, dtype=v.dtype if len(v) else np.int16)
    buf[: len(v)] = v
    return buf.reshape(ncols, 16).T.copy()


def prep(x, edge_index, edge_weight, batch):
    """Pure index/permutation prep. Returns (per-core input dicts, meta)."""
    src = np.asarray(edge_index[0], dtype=np.int64)
    dst = np.asarray(edge_index[1], dtype=np.int64)
    w = np.asarray(edge_weight, dtype=np.float32)
    batch = np.asarray(batch, dtype=np.int64)
    x = np.asarray(x, dtype=np.float32)

    g_all = src // NPC
    core_all = dst // NPC
    chunk_all = (dst % NPC) // M
    cell = ((core_all * NC + g_all) * S + chunk_all).astype(np.int64)
    C_ch = _ceil16(int(np.bincount(cell, minlength=NC * NC * S).max()) + 2)
    loops_dst = np.concatenate([dst, np.arange(N, dtype=np.int64)])
    DP = int(np.bincount(loops_dst, minlength=N).max())

    cnt = np.maximum(np.bincount(batch, minlength=B), 1).astype(np.int32)

    sel = np.zeros((128, 16), dtype=np.float16)
    sel[np.arange(128), np.arange(128) % 16] = 1.0
    sel2 = np.zeros((128, 1), dtype=np.float16)
    sel2[::16, 0] = 1.0

    cores = []
    for c in range(NC):
        lo = c * NPC
        hi = min((c + 1) * NPC, N)
        nreal = hi - lo

        xpk = np.zeros((128, NBLK * 11), dtype=np.float16)
        xl = np.zeros((NPC, F), dtype=np.float32)
        xl[:nreal] = x[lo:hi]
        for b_ in range(NBLK):
            xpk[:, b_ * 11 + 1 : b_ * 11 + 1 + F] = xl[b_ * 128 : (b_ + 1) * 128]

        mask = (dst >= lo) & (dst < hi)
        es, ed, ew = src[mask], (dst[mask] - lo).astype(np.int64), w[mask]
        eg = es // NPC
        esl = (es - eg * NPC).astype(np.int16)

        # deg layout includes self-loops (w=1) and pad nodes (w=1)
        w2pad = np.zeros((128, NBLK * DP), dtype=np.float32)
        order_d = np.argsort(ed, kind="stable")
        d_sorted, w_sorted = ed[order_d], ew[order_d]
        node_starts = np.searchsorted(d_sorted, np.arange(NPC + 1))
        p_of = np.arange(NPC) % 128
        b_of = np.arange(NPC) // 128
        lens = np.diff(node_starts)
        for l in np.nonzero(lens)[0]:
            a = node_starts[l]
            w2pad[p_of[l], b_of[l] * DP + 1 : b_of[l] * DP + 1 + lens[l]] = (
                w_sorted[a : a + lens[l]]
            )
        w2pad[p_of, b_of * DP] = 1.0  # self loop (and pad-node deg 1)

        gidx = np.zeros((128, S * (C_ch // 16)), dtype=np.int16)
        wrep = np.zeros((128, S * C_ch), dtype=np.float16)
        eidx = np.zeros((128, S * ECOLS), dtype=np.uint16)
        order = np.lexsort((ed, eg))
        gs, ds, ws, sls = eg[order], ed[order], ew[order], esl[order]
        grp_starts = np.searchsorted(gs, np.arange(NC + 1))
        for k in range(NC):
            ga, gb = grp_starts[k], grp_starts[k + 1]
            dk, wk, slk = ds[ga:gb], ws[ga:gb], sls[ga:gb]
            chunk_starts = np.searchsorted(dk, np.arange(0, NPC + M, M))
            for j in range(S):
                ca, cb = chunk_starts[j], chunk_starts[j + 1]
                n_e = cb - ca
                idx_slots = np.zeros(C_ch, dtype=np.int16)
                idx_slots[1 : 1 + n_e] = slk[ca:cb]
                w_slots = np.zeros(C_ch, dtype=np.float16)
                w_slots[1 : 1 + n_e] = wk[ca:cb]
                gidx[16 * k : 16 * (k + 1), j * (C_ch // 16) : (j + 1) * (C_ch // 16)] = (
                    idx_slots.reshape(C_ch // 16, 16).T
                )
                wrep[16 * k : 16 * (k + 1), j * C_ch : (j + 1) * C_ch] = w_slots[None, :]
                ends = np.zeros(M + 1, dtype=np.uint16)
                ends[1:] = np.searchsorted(
                    dk[ca:cb], np.arange(j * M, (j + 1) * M), side="right"
                ).astype(np.uint16)
                epad = np.zeros(ECOLS * 16, dtype=np.uint16)
                epad[: M + 1] = ends
                eidx[16 * k : 16 * (k + 1), j * ECOLS : (j + 1) * ECOLS] = epad.reshape(
                    ECOLS, 16
                ).T

        # one-hot selectors for self-loop adds (per-core data, shared program)
        selfadd = np.zeros((128, F), dtype=np.float32)
        selfadd[16 * c + np.arange(F), np.arange(F)] = 1.0
        selfsel = np.zeros((128, 1), dtype=np.float32)
        selfsel[16 * c, 0] = 1.0

        # pooling: local graph slots + 0/1 indicators per 128-node block
        gmin, gmax = int(batch[lo]), int(batch[hi - 1])
        n_gc = gmax - gmin + 1
        assert n_gc <= GCOL
        ind = np.zeros((128, NBLK * GCOL), dtype=np.float16)
        lslot = (batch[lo:hi] - gmin).astype(np.int64)  # local graph slot/node
        nn = np.arange(nreal)
        ind[p_of[:nreal], b_of[:nreal] * GCOL + lslot] = 1.0

        cores.append(
            dict(
                xpk=xpk, w2pad=w2pad, gidx=gidx, wrep=wrep, eidx=eidx,
                selt=sel, sel2=sel2, selfadd=selfadd, selfsel=selfsel,
                ind=ind, cnt=cnt.reshape(1, B),
                gmin=gmin, n_gc=n_gc,
            )
        )

    # combine maps (same for every core; only core 0's output is read)
    idx1 = np.full(B, GTOT - 1, dtype=np.int16)  # zero slot
    idx2 = np.full(B, GTOT - 1, dtype=np.int16)
    for c in range(NC):
        gmin, n_gc = cores[c]["gmin"], cores[c]["n_gc"]
        for sl in range(n_gc):
            g = gmin + sl
            if idx1[g] == GTOT - 1:
                idx1[g] = c * GCOL + sl
            else:
                idx2[g] = c * GCOL + sl
    cmb1 = _wrap16(idx1, B // 16)
    cmb2 = _wrap16(idx2, B // 16)
    for c in range(NC):
        cores[c]["cmb1"] = cmb1.astype(np.int16)
        cores[c]["cmb2"] = cmb2.astype(np.int16)
        del cores[c]["gmin"], cores[c]["n_gc"]
    return cores, dict(C_ch=C_ch, DP=DP)


# ------------------------------------------------------------------ device
def build_program(C_ch, DP):
    import concourse.bass as bass
    import concourse.bacc as bacc
    import concourse.mybir as mybir
    import concourse.tile as tile
    from concourse.masks import make_identity

    f32 = mybir.dt.float32
    f32r = mybir.dt.float32r
    f16 = mybir.dt.float16
    i16 = mybir.dt.int16
    u16 = mybir.dt.uint16
    i32 = mybir.dt.int32
    AX = mybir.AxisListType.X
    OP = mybir.AluOpType
    AF = mybir.ActivationFunctionType

    nc = bacc.Bacc("TRN2", target_bir_lowering=False, debug=False, num_devices=NC)

    def din(name, shape, dt=f16):
        return nc.dram_tensor(name, shape, dt, kind="ExternalInput")

    xpk_d = din("xpk", [128, NBLK * 11])
    w2_d = din("w2pad", [128, NBLK * DP], f32)
    gidx_d = din("gidx", [128, S * (C_ch // 16)], i16)
    wrep_d = din("wrep", [128, S * C_ch])
    eidx_d = din("eidx", [128, S * ECOLS], u16)
    selt_d = din("selt", [128, 16])
    sel2_d = din("sel2", [128, 1])
    selfadd_d = din("selfadd", [128, F], f32)
    selfsel_d = din("selfsel", [128, 1], f32)
    ind_d = din("ind", [128, NBLK * GCOL])
    cmb1_d = din("cmb1", [16, B // 16], i16)
    cmb2_d = din("cmb2", [16, B // 16], i16)
    cnt_d = din("cnt", [1, B], i32)
    W1_d = din("W1", [F, 128], f32)
    b1_d = din("b1", [128, 1], f32)
    W2_d = din("W2", [128, 128], f32)
    wlr_d = din("wlin_row", [1, 128], f32)
    wlc_d = din("wlin_col", [128, 1], f32)
    blin_d = din("blin", [1, 1], f32)
    b2_d = din("b2row", [1, 128], f32)
    out_d = nc.dram_tensor("out", [1, B], f32, kind="ExternalOutput")

    rg = [list(range(NC))]
    GC = C_ch // 16

    with tile.TileContext(nc) as tc:
        from contextlib import ExitStack

        with ExitStack() as ctx:
            sb = ctx.enter_context(tc.tile_pool(name="sb", bufs=1))
            big = ctx.enter_context(tc.tile_pool(name="big", bufs=1))
            dram = ctx.enter_context(tc.tile_pool(name="dram", bufs=1, space="DRAM"))
            mpool = ctx.enter_context(tc.tile_pool(name="mp", bufs=1))
            scpool = ctx.enter_context(tc.tile_pool(name="scp", bufs=2))
            wpool = ctx.enter_context(tc.tile_pool(name="wp", bufs=1))
            dpool = ctx.enter_context(tc.tile_pool(name="dvb", bufs=1))
            zwpool = ctx.enter_context(tc.tile_pool(name="zw", bufs=2))
            epool = ctx.enter_context(tc.tile_pool(name="ep", bufs=2))
            ppool = ctx.enter_context(tc.tile_pool(name="pp", bufs=2))
            tpool = ctx.enter_context(tc.tile_pool(name="tp", bufs=2))
            hpool = ctx.enter_context(tc.tile_pool(name="hp", bufs=2))

            # --- constants / inputs resident in SBUF
            selt = sb.tile([128, 16], f16)
            nc.sync.dma_start(out=selt[:], in_=selt_d[:, :])
            sel2t = sb.tile([128, 1], f16)
            nc.sync.dma_start(out=sel2t[:], in_=sel2_d[:, :])
            selfaddt = sb.tile([128, F], f32)
            nc.sync.dma_start(out=selfaddt[:], in_=selfadd_d[:, :])
            selfselt = sb.tile([128, 1], f32)
            nc.sync.dma_start(out=selfselt[:], in_=selfsel_d[:, :])
            b1t = sb.tile([128, 1], f32)
            nc.sync.dma_start(out=b1t[:], in_=b1_d[:, :])
            wlrt = sb.tile([1, 128], f32)
            nc.sync.dma_start(out=wlrt[:], in_=wlr_d[:, :])
            blint = sb.tile([1, 1], f32)
            nc.sync.dma_start(out=blint[:], in_=blin_d[:, :])
            b2t = sb.tile([1, 128], f32)
            nc.sync.dma_start(out=b2t[:], in_=b2_d[:, :])
            cntt = sb.tile([1, B], i32)
            nc.sync.dma_start(out=cntt[:], in_=cnt_d[:, :])
            cmb1t = sb.tile([16, B // 16], i16)
            nc.sync.dma_start(out=cmb1t[:], in_=cmb1_d[:, :])
            cmb2t = sb.tile([16, B // 16], i16)
            nc.sync.dma_start(out=cmb2t[:], in_=cmb2_d[:, :])
            zerot = sb.tile([128, 1], f32)
            nc.vector.memset(zerot[:], 0.0)
            zerot16 = sb.tile([128, 1], f16)
            nc.vector.memset(zerot16[:], 0.0)

            # resident edge indices (shared by both layers)
            gidx = big.tile([128, S * GC], i16)
            nc.sync.dma_start(out=gidx[:], in_=gidx_d[:, :])
            eidx = big.tile([128, S * ECOLS], u16)
            nc.sync.dma_start(out=eidx[:], in_=eidx_d[:, :])

            dinv32 = sb.tile([128, NBLK], f32)   # transposed layout [p, blk]
            wz16 = sb.tile([128, 1], f16)
            w1t16 = sb.tile([F, 128], f16)
            table = big.tile([128, NPC], f32)

            dr_dinv = dram.tile([1, NPC], f16)
            yag_in = dram.tile([F, NPC], f16)
            yag_out = dram.tile([NC, F * NPC], f16)

            # --- phase A: deg, dinv, y, transposes, wz
            with tc.tile_pool(name="pha", bufs=1) as pha, \
                 tc.tile_pool(name="pst", bufs=2, space="PSUM") as pst:
                ident = pha.tile([128, 128], f16)
                make_identity(nc, ident[:])
                identf = pha.tile([128, 128], f32)
                make_identity(nc, identf[:])
                W2t = pha.tile([128, 128], f32)
                nc.sync.dma_start(out=W2t[:], in_=W2_d[:, :])
                wlct = pha.tile([128, 1], f32)
                nc.sync.dma_start(out=wlct[:], in_=wlc_d[:, :])
                w2tp = pst.tile([128, 512], f32, tag="pt")
                nc.tensor.transpose(out=w2tp[:, :128], in_=W2t[:], identity=identf[:])
                w2ts = pha.tile([128, 128], f32)
                nc.scalar.copy(out=w2ts[:], in_=w2tp[:, :128])
                wzp = pst.tile([128, 512], f32, tag="pt")
                nc.tensor.matmul(out=wzp[:, :1], lhsT=w2ts[:], rhs=wlct[:],
                                 start=True, stop=True)
                nc.scalar.copy(out=wz16[:], in_=wzp[:, :1])
                W1t = pha.tile([F, 128], f32)
                nc.sync.dma_start(out=W1t[:], in_=W1_d[:, :])
                nc.scalar.copy(out=w1t16[:], in_=W1t[:])

                # deg from w2pad (includes self loops)
                deg = pha.tile([128, NBLK], f32)
                w2t_ = pha.tile([128, NBLK * DP], f32)
                nc.sync.dma_start(out=w2t_[:], in_=w2_d[:, :])
                nc.vector.tensor_reduce(
                    out=deg[:], in_=w2t_[:].rearrange("p (b d) -> p b d", d=DP),
                    axis=AX, op=OP.add,
                )
                nc.scalar.activation(out=deg[:], in_=deg[:], func=AF.Sqrt)
                nc.vector.reciprocal(out=dinv32[:], in_=deg[:])
                dinv16T = pha.tile([128, NBLK], f16)
                nc.vector.tensor_copy(out=dinv16T[:], in_=dinv32[:])

                # y = x * dinv (fp16), slot 0 <- dinv
                xpkt = pha.tile([128, NBLK * 11], f16)
                nc.sync.dma_start(out=xpkt[:], in_=xpk_d[:, :])
                xv = xpkt[:].rearrange("p (b f) -> p b f", f=11)
                dv = dinv16T[:].rearrange("p (b o) -> p b o", o=1)
                nc.vector.tensor_tensor(
                    out=xv[:, :, 1 : F + 1], in0=xv[:, :, 1 : F + 1],
                    in1=dv.to_broadcast([128, NBLK, 11 - 1])[:, :, : F], op=OP.mult,
                )
                nc.vector.tensor_copy(out=xv[:, :, 0:1], in_=dv)

                # transpose -> yt [11, NPC] fp16
                yt = pha.tile([11, NPC], f16)
                for b4 in range(25):
                    nb = min(4, NBLK - b4 * 4)
                    ptile = pst.tile([128, 512], f16, tag="pt16", name=f"pt{b4}")
                    for bb in range(nb):
                        b_ = b4 * 4 + bb
                        nc.tensor.transpose(
                            out=ptile[0:11, bb * 128 : (bb + 1) * 128],
                            in_=xv[:, b_, :], identity=ident[:],
                        )
                    nc.scalar.copy(
                        out=yt[0:11, b4 * 512 : b4 * 512 + nb * 128],
                        in_=ptile[0:11, : nb * 128],
                    )
                nc.sync.dma_start(out=dr_dinv[:], in_=yt[0:1, :])
                nc.sync.dma_start(out=yag_in[:], in_=yt[1 : F + 1, :])

            # --- AllGather y (fp16); table rows 16k+f
            nc.gpsimd.collective_compute(
                "AllGather", mybir.AluOpType.bypass, replica_groups=rg,
                ins=[yag_in[:]], outs=[yag_out[:]],
            )
            yag_v = yag_out[:].rearrange("k (f n) -> k f n", f=F)
            with tc.tile_pool(name="stg", bufs=1) as stg:
                stg16 = stg.tile([128, NPC], f16)
                nc.vector.memset(stg16[:], 0.0)
                for k in range(NC):
                    nc.sync.dma_start(out=stg16[16 * k : 16 * k + F, :], in_=yag_v[k])
                nc.scalar.copy(out=table[:], in_=stg16[:])

            mpool = ctx.enter_context(tc.tile_pool(name="mp", bufs=1))
            scpool = ctx.enter_context(tc.tile_pool(name="scp", bufs=2))
            wpool = ctx.enter_context(tc.tile_pool(name="wp", bufs=1))
            dpool = ctx.enter_context(tc.tile_pool(name="dvb", bufs=1))
            zwpool = ctx.enter_context(tc.tile_pool(name="zw", bufs=2))
            epool = ctx.enter_context(tc.tile_pool(name="ep", bufs=2))
            ppool = ctx.enter_context(tc.tile_pool(name="pp", bufs=2))
            tpool = ctx.enter_context(tc.tile_pool(name="tp", bufs=2))
            hpool = ctx.enter_context(tc.tile_pool(name="hp", bufs=2))
            psm = ctx.enter_context(tc.tile_pool(name="psm", bufs=2, space="PSUM"))
            psb = ctx.enter_context(tc.tile_pool(name="psb", bufs=2, space="PSUM"))
            psq = ctx.enter_context(tc.tile_pool(name="psq", bufs=1, space="PSUM"))

            def edge_win(j0, nch, lyr):
                msgs = mpool.tile([128, 7 * C_ch], f32, tag="msgs", name=f"m{lyr}_{j0}")
                nc.gpsimd.ap_gather(
                    out_ap=msgs[:, : nch * C_ch], in_ap=table[:],
                    idxs_ap=gidx[:, j0 * GC : (j0 + nch) * GC],
                    channels=128, num_elems=NPC, d=1, num_idxs=nch * C_ch,
                )
                wre = wpool.tile([128, 7 * C_ch], f16, tag="wre", name=f"w{lyr}_{j0}")
                nc.sync.dma_start(
                    out=wre[:, : nch * C_ch],
                    in_=wrep_d[:, j0 * C_ch : (j0 + nch) * C_ch],
                )
                return msgs, wre

            def chunk_pd(mw, js, j, lyr):
                msgs, wre = mw
                sc32 = scpool.tile([128, C_ch], f32, tag="sc", name=f"s{lyr}_{j}")
                nc.vector.tensor_tensor(
                    out=sc32[:], in0=msgs[:, js * C_ch : (js + 1) * C_ch],
                    in1=wre[:, js * C_ch : (js + 1) * C_ch], op=OP.mult,
                )
                sc = sc32[:]
                nc.vector.tensor_tensor_scan(
                    out=sc, data0=sc,
                    data1=zerot[:].to_broadcast([128, C_ch]),
                    initial=0.0, op0=OP.add, op1=OP.add,
                )
                E = epool.tile([128, 904], f32, tag="E", name=f"E{lyr}_{j}")
                nc.gpsimd.indirect_copy(
                    out=E[:, : M + 1], data=sc,
                    idxs=eidx[:, j * ECOLS : (j + 1) * ECOLS],
                    i_know_ap_gather_is_preferred=True,
                )
                Pd = ppool.tile([128, M], f16, tag="Pd", name=f"Pd{lyr}_{j}")
                nc.vector.tensor_tensor(
                    out=Pd[:], in0=E[:, 1 : M + 1], in1=E[:, 0:M], op=OP.subtract
                )
                return Pd

            # --- layer 1
            zag_in = dram.tile([1, NPC], f32)
            for (j0, nch) in WINS:
                mw = edge_win(j0, nch, 1)
                dinvb = dpool.tile([F, 7 * M], f16, tag="dvb", name=f"dvb{j0}")
                for p in range(F):
                    nc.sync.dma_start(
                        out=dinvb[p : p + 1, : nch * M],
                        in_=dr_dinv[:, j0 * M : (j0 + nch) * M],
                    )
                for js in range(nch):
                    j = j0 + js
                    zyw = zwpool.tile([1, M], f32, tag="zyw", name=f"zyw{j}")
                    Pd = chunk_pd(mw, js, j, 1)
                    pT = tpool.tile([F, M], f16, tag="pT")
                    for h in range(2):
                        pm = psm.tile([F, 448], f32, tag="pm")
                        nc.tensor.matmul(
                            out=pm[:], lhsT=selt[:, 0:F],
                            rhs=Pd[:, h * 448 : (h + 1) * 448],
                            start=True, stop=False,
                        )
                        nc.tensor.matmul(
                            out=pm[:], lhsT=selfaddt[:],
                            rhs=table[:, j * M + h * 448 : j * M + (h + 1) * 448],
                            start=False, stop=True,
                        )
                        nc.vector.tensor_tensor(
                            out=pT[:, h * 448 : (h + 1) * 448], in0=pm[:],
                            in1=dinvb[:, js * M + h * 448 : js * M + (h + 1) * 448],
                            op=OP.mult,
                        )
                    for h in range(2):
                        st = psb.tile([128, 448], f32, tag="st")
                        nc.tensor.matmul(
                            out=st[:], lhsT=w1t16[:],
                            rhs=pT[:, h * 448 : (h + 1) * 448],
                            start=True, stop=True,
                        )
                        ht = hpool.tile([128, 448], f16, tag="ht")
                        nc.scalar.activation(out=ht[:], in_=st[:], func=AF.Relu,
                                             bias=b1t[:])
                        zp = psm.tile([1, 448], f32, tag="zp")
                        nc.tensor.matmul(
                            out=zp[:], lhsT=wz16[:], rhs=ht[:], start=True, stop=True
                        )
                        nc.vector.tensor_tensor(
                            out=zyw[0:1, h * 448 : (h + 1) * 448],
                            in0=zp[:],
                            in1=dinvb[0:1, js * M + h * 448 : js * M + (h + 1) * 448],
                            op=OP.mult,
                        )
                    nc.sync.dma_start(
                        out=zag_in[:, j * M : (j + 1) * M], in_=zyw[0:1, :]
                    )

            # --- AllGather zy; overwrite table rows 16k
            zag_out = dram.tile([NC, NPC], f32)
            nc.gpsimd.collective_compute(
                "AllGather", mybir.AluOpType.bypass, replica_groups=rg,
                ins=[zag_in[:]], outs=[zag_out[:]],
            )
            for k in range(NC):
                nc.sync.dma_start(out=table[16 * k : 16 * k + 1, :],
                                  in_=zag_out[k : k + 1, :])

            # --- layer 2 + pooling accumulation
            indt = big.tile([128, NBLK * GCOL], f16)
            nc.sync.dma_start(out=indt[:], in_=ind_d[:, :])
            qtp = psq.tile([128, NBLK], f32, tag="qtp")
            gsum = psq.tile([1, GCOL], f32, tag="gsum")
            q98 = sb.tile([128, NBLK], f16)
            for (j0, nch) in WINS:
                mw = edge_win(j0, nch, 2)
                for js in range(nch):
                    j = j0 + js
                    Pd = chunk_pd(mw, js, j, 2)
                    for bb in range(7):
                        col = j * 7 + bb
                        nc.tensor.matmul(
                            out=qtp[:, col : col + 1],
                            lhsT=Pd[:, bb * 128 : (bb + 1) * 128],
                            rhs=sel2t[:], start=True, stop=False,
                        )
                        nc.tensor.matmul(
                            out=qtp[:, col : col + 1],
                            lhsT=table[:, col * 128 : (col + 1) * 128],
                            rhs=selfselt[:], start=False, stop=True,
                        )
                    nc.vector.tensor_tensor(
                        out=q98[:, j * 7 : (j + 1) * 7],
                        in0=qtp[:, j * 7 : (j + 1) * 7],
                        in1=dinv32[:, j * 7 : (j + 1) * 7], op=OP.mult,
                    )
                    for bb in range(7):
                        col = j * 7 + bb
                        nc.tensor.matmul(
                            out=gsum[:], lhsT=q98[:, col : col + 1],
                            rhs=indt[:, col * GCOL : (col + 1) * GCOL],
                            start=(col == 0), stop=(col == NBLK - 1),
                        )

            # --- final combine on AllGathered per-core sums
            g32 = sb.tile([1, GCOL], f32)
            nc.vector.tensor_copy(out=g32[:], in_=gsum[:])
            gin = dram.tile([1, GCOL], f32)
            gout = dram.tile([1, NC * GCOL], f32)
            nc.sync.dma_start(out=gin[:], in_=g32[:])
            nc.gpsimd.collective_compute(
                "AllGather", mybir.AluOpType.bypass, replica_groups=rg,
                ins=[gin[:]], outs=[gout[:]],
            )
            gall = sb.tile([16, GTOT], f32)
            nc.vector.memset(gall[:], 0.0)
            nc.sync.dma_start(out=gall[0:1, : NC * GCOL], in_=gout[:, :])
            ga = sb.tile([16, B], f32)
            nc.gpsimd.ap_gather(
                out_ap=ga[:], in_ap=gall[:], idxs_ap=cmb1t[:],
                channels=16, num_elems=GTOT, d=1, num_idxs=B,
            )
            gb = sb.tile([16, B], f32)
            nc.gpsimd.ap_gather(
                out_ap=gb[:], in_ap=gall[:], idxs_ap=cmb2t[:],
                channels=16, num_elems=GTOT, d=1, num_idxs=B,
            )
            nc.vector.tensor_tensor(out=ga[0:1, :], in0=ga[0:1, :], in1=gb[0:1, :],
                                    op=OP.add)
            cntf = sb.tile([1, B], f32)
            nc.vector.tensor_copy(out=cntf[:], in_=cntt[:])
            rec = sb.tile([1, B], f32)
            nc.vector.reciprocal(out=rec[:], in_=cntf[:])
            res = sb.tile([1, B], f32)
            nc.vector.tensor_tensor(out=res[:], in0=ga[0:1, :], in1=rec[:], op=OP.mult)
            cb = sb.tile([1, 128], f32)
            nc.vector.tensor_tensor(out=cb[:], in0=b2t[:], in1=wlrt[:], op=OP.mult)
            cs = sb.tile([1, 1], f32)
            nc.vector.tensor_reduce(out=cs[:], in_=cb[:], axis=AX, op=OP.add)
            nc.vector.tensor_tensor(out=cs[:], in0=cs[:], in1=blint[:], op=OP.add)
            nc.vector.tensor_tensor(
                out=res[:], in0=res[:], in1=cs[:].to_broadcast([1, B]), op=OP.add
            )
            nc.sync.dma_start(out=out_d[:, :], in_=res[:])

    nc.compile()
    return nc


_CACHE = {}


def kernel(**inputs):
    from concourse.bass_utils import run_bass_kernel_spmd

    cores, meta = prep(
        inputs["x"], inputs["edge_index"], inputs["edge_weight"], inputs["batch"]
    )
    key = (meta["C_ch"], meta["DP"])
    if key not in _CACHE:
        _CACHE[key] = build_program(*key)
    nc = _CACHE[key]

    W1 = np.asarray(inputs["W1"], dtype=np.float32)
    b1 = np.asarray(inputs["b1"], dtype=np.float32).reshape(128, 1)
    W2 = np.asarray(inputs["W2"], dtype=np.float32)
    wlr = np.asarray(inputs["Wlin"], dtype=np.float32).reshape(1, 128)
    wlc = np.asarray(inputs["Wlin"], dtype=np.float32).reshape(128, 1)
    blin = np.asarray(inputs["blin"], dtype=np.float32).reshape(1, 1)
    b2r = np.asarray(inputs["b2"], dtype=np.float32).reshape(1, 128)

    in_maps = []
    for c in range(NC):
        cr = cores[c]
        in_maps.append(
            dict(
                xpk=cr["xpk"], w2pad=cr["w2pad"], gidx=cr["gidx"], wrep=cr["wrep"],
                eidx=cr["eidx"], selt=cr["selt"], sel2=cr["sel2"],
                selfadd=cr["selfadd"], selfsel=cr["selfsel"], ind=cr["ind"],
                cmb1=cr["cmb1"], cmb2=cr["cmb2"], cnt=cr["cnt"],
                W1=W1, b1=b1, W2=W2, wlin_row=wlr, wlin_col=wlc, blin=blin, b2row=b2r,
            )
        )
    res = run_bass_kernel_spmd(nc, in_maps, list(range(NC)))
    out = np.asarray(res.results[0]["out"], dtype=np.float32).reshape(B, 1)
    return out
